# revision 25
# baseline (speedup 1.0000x reference)
"""Bass/Trainium2 kernel for nn_FC_Classifier (box-pooled FC classifier).

Math: pred[n,k] = (1/area_n) * sum_{(h,w) in box_n} (fc_w @ feature_map)[k,h,w] + fc_b[k]

Strategy (8 cores, one chip), v3:
  * Shard image rows h across cores (24 rows/core).  Phase 1 contracts
    channels (2048 -> 150) with matmuls (host-swizzled fm, contiguous DMA).
  * W-cumsum fused per image row via triangular matmul (bf16 tri, f32 PSUM).
  * H-cumsum fused into the PSUM->SBUF copies as a running add chain.
  * Tiny AllGather of per-block column totals + mask-weighted prefix sum.
  * Integral image AllGather split into h-chunks (separate Shared tensors);
    anchors are grouped so every corner-gather instruction reads exactly one
    chunk tensor, so gathers overlap the later AllGather chunks' wire time.
  * 4-corner indirect-DMA gathers, combine on DVE, scale by 1/area; bias on
    host.

Self-contained: only numpy + the concourse (Bass) runtime are imported.
"""

import os
import numpy as np

DS = 8.0
NCORES = 8
C, H, W, K, N_ANCH = 2048, 192, 192, 150, 16384
HSH = H // NCORES              # 24 image rows per core
XP = 200                       # x range of S (0..192 used), padded to 8*25
CCH = C // 128                 # 16 channel chunks
HQ = 12                        # fm DMA chunks (2 rows each)
HR = HSH // HQ

LAST_RESULTS = None  # BassKernelResults of the most recent run (for test.py)

_NC_CACHE = {}


def _chunks(total, size):
    return [(o, min(size, total - o)) for o in range(0, total, size)]


def _chunk_list():
    s = os.environ.get("NMS_AG_CHUNKS", "24")
    ch = [int(x) for x in s.split(",") if x]
    assert sum(ch) == HSH, ch
    return ch


def _box_indices_np(anchors, scale, h, w):
    # exact replica of reference._box_indices in numpy f32
    a = anchors.astype(np.float32) / np.float32(DS)
    x0 = (a[:, 0] * scale[1]).astype(np.int32)
    x1 = (a[:, 1] * scale[1]).astype(np.int32)
    y0 = (a[:, 2] * scale[0]).astype(np.int32)
    y1 = (a[:, 3] * scale[0]).astype(np.int32)
    eqy = y0 == y1
    y0, y1 = (
        np.where(eqy & (y0 != 0), y0 - 1, y0),
        np.where(eqy & (y0 == 0), y1 + 1, y1),
    )
    eqx = x0 == x1
    x0, x1 = (
        np.where(eqx & (x0 != 0), x0 - 1, x0),
        np.where(eqx & (x0 == 0), x1 + 1, x1),
    )
    y0, y1 = np.clip(y0, 0, h), np.clip(y1, 0, h)
    x0, x1 = np.clip(x0, 0, w), np.clip(x1, 0, w)
    return x0, x1, y0, y1


def _build_nc(CH, plan):
    """Build the SPMD Bass program (identical on all 8 cores).

    CH: h-chunk sizes for the big AllGather (sum = 24).
    plan: per gather batch, (a, b) = chunk tensor ids for the y1-corners
          (j0/j2) and y0-corners (j1/j3).
    """
    from concourse import bacc, mybir, tile
    import concourse.bass as bass

    f32 = mybir.dt.float32
    bf16 = mybir.dt.bfloat16
    i32 = mybir.dt.int32

    wch = _chunks(W, 128)          # [(0,128),(128,64)]   w partition chunks
    xch = _chunks(XP, 128)         # [(0,128),(128,72)]   x partition chunks
    HO = np.concatenate([[0], np.cumsum(CH)]).astype(int)
    nch = len(CH)
    NB = plan[0] + plan[1]

    nc = bacc.Bacc("TRN2", target_bir_lowering=False, debug=False,
                   num_devices=NCORES)
    # host-swizzled fm: [p, hq, cc, hr, w] so each h-chunk DMA is contiguous
    fm = nc.dram_tensor("fm", [128, HQ, CCH, HR, W], bf16, kind="ExternalInput").ap()
    fcw = nc.dram_tensor("fcw", [128, CCH, K], bf16, kind="ExternalInput").ap()
    trib = nc.dram_tensor("trib", [W, XP], bf16, kind="ExternalInput").ap()
    cidx = nc.dram_tensor("cidx", [4, 128, NB], i32, kind="ExternalInput").ap()
    iar = nc.dram_tensor("iar", [128, NB], f32, kind="ExternalInput").ap()
    mask = nc.dram_tensor("mask", [128, NCORES], f32, kind="ExternalInput").ap()
    pred = nc.dram_tensor("pred", [128 * NB, K], f32, kind="ExternalOutput").ap()

    RG = [list(range(NCORES))]
    NF = HSH * K

    with tile.TileContext(nc) as tc:
        with (
            tc.tile_pool(name="constp", bufs=1) as constp,
            tc.tile_pool(name="fmp", bufs=3) as fmp,
            tc.tile_pool(name="gp", bufs=3) as gp,
            tc.tile_pool(name="qp", bufs=1) as qp,
            tc.tile_pool(name="psp", bufs=2, space="PSUM") as psp,
            tc.tile_pool(name="gatp", bufs=12) as gatp,
            tc.tile_pool(name="xbp", bufs=1) as xbp,
            tc.tile_pool(name="dramp", bufs=1, space="DRAM") as dramp,
        ):
            # ---- constants -------------------------------------------------
            fcw_sb = constp.tile([128, CCH * K], bf16, tag="fcw", name="fcw_sb")
            nc.sync.dma_start(fcw_sb[:], fcw.rearrange("p cc k -> p (cc k)"))

            tri_w = []                       # [wsz, XP] per w-chunk
            for j, (off, sz) in enumerate(wch):
                t = constp.tile([sz, XP], bf16, tag=f"tri_w{j}", name=f"tri_w{j}")
                nc.sync.dma_start(t[:], trib[off:off + sz, :])
                tri_w.append(t)

            idx_sb = constp.tile([128, 4 * NB], i32, tag="idx", name="idx_sb")
            nc.sync.dma_start(idx_sb[:], cidx.rearrange("c p m -> p c m"))
            iar_sb = constp.tile([128, NB], f32, tag="iar", name="iar_sb")
            nc.sync.dma_start(iar_sb[:], iar[:, :])
            mask_sb = constp.tile([128, NCORES], f32, tag="mask", name="mask_sb")
            nc.sync.dma_start(mask_sb[:], mask[:, :])

            # persistent Q-cumsum buffers [x, (h k)]
            Qc = [qp.tile([sz, NF], f32, tag=f"Qc{j}", name=f"Qc{j}")
                  for j, (off, sz) in enumerate(xch)]

            # warm up the collective path early so the first real collective
            # doesn't pay the ncfw entry latency
            warm_in = dramp.tile([128, 4], f32, tag="warm_in", name="warm_in")
            warm_out = dramp.tile([128, 4], f32, tag="warm_out",
                                  name="warm_out", addr_space="Shared")
            wsb = constp.tile([128, 4], f32, tag="warm_sb", name="warm_sb")
            nc.gpsimd.memset(wsb[:], 0.0)
            nc.sync.dma_start(warm_in[:, :], wsb[:])
            nc.gpsimd.collective_compute(
                "AllReduce", mybir.AluOpType.add, replica_groups=RG,
                ins=[warm_in[:].opt()], outs=[warm_out[:].opt()],
            )

            # ---- phase 1: projection + W-cumsum + fused H-cumsum -----------
            for hq in range(HQ):
                fmh = fmp.tile([128, CCH * HR * W], bf16, tag="fmh", name="fmh")
                nc.sync.dma_start(fmh[:], fm.rearrange("p hq cc hr w -> p hq (cc hr w)")[:, hq])
                for hr in range(HR):
                    h = hq * HR + hr
                    gts = []
                    for wj, (woff, wsz) in enumerate(wch):
                        ps = psp.tile([wsz, K], f32, tag=f"pp{wj}", name="ps1")
                        for cc in range(CCH):
                            o = cc * (HR * W) + hr * W + woff
                            nc.tensor.matmul(
                                ps[:],
                                lhsT=fmh[:, o: o + wsz],
                                rhs=fcw_sb[:, cc * K:(cc + 1) * K],
                                start=(cc == 0), stop=(cc == CCH - 1),
                            )
                        gt = gp.tile([wsz, K], bf16, tag=f"g{wj}", name=f"g{wj}")
                        nc.vector.tensor_copy(gt[:], ps[:])
                        gts.append(gt)
                    for xj, (xoff, xsz) in enumerate(xch):
                        qs = psp.tile([xsz, K], f32, tag=f"wp{xj}", name="ps2")
                        for wj in range(len(wch)):
                            nc.tensor.matmul(
                                qs[:],
                                lhsT=tri_w[wj][:, xoff:xoff + xsz],
                                rhs=gts[wj][:],
                                start=(wj == 0), stop=(wj == len(wch) - 1),
                            )
                        if h == 0:
                            nc.vector.tensor_copy(Qc[xj][:, 0:K], qs[:])
                        else:
                            nc.vector.tensor_add(
                                Qc[xj][:, h * K:(h + 1) * K], qs[:],
                                Qc[xj][:, (h - 1) * K:h * K])

            # ---- block totals exchange (replicated AllToAll = AllGather) ---
            ag2_in = dramp.tile([NCORES * XP, K], f32, tag="ag2_in",
                                name="ag2_in")
            ag2_out = dramp.tile([NCORES * XP, K], f32, tag="ag2_out",
                                 name="ag2_out")
            a2iv = ag2_in.rearrange("(b x) k -> x b k", b=NCORES)
            for xj, (xoff, xsz) in enumerate(xch):
                src = (Qc[xj][:, (HSH - 1) * K:HSH * K]
                       .unsqueeze(1).broadcast_to([xsz, NCORES, K]))
                nc.sync.dma_start(a2iv[xoff:xoff + xsz], src)
            nc.gpsimd.collective_compute(
                "AllToAll", mybir.AluOpType.bypass, replica_groups=RG,
                ins=[ag2_in[:].opt()], outs=[ag2_out[:].opt()],
            )
            a2v = ag2_out.rearrange("(b x) k -> x b k", b=NCORES)
            P = []
            for xj, (xoff, xsz) in enumerate(xch):
                tall = qp.tile([xsz, NCORES * K], f32, tag=f"tall{xj}",
                               name=f"tall{xj}")
                nc.sync.dma_start(
                    tall[:].rearrange("x (b k) -> x b k", b=NCORES),
                    a2v[xoff:xoff + xsz])
                # tree-structured masked sum: 4 fused ops + 3 adds
                pairs = []
                for b in range(0, NCORES, 2):
                    t = qp.tile([xsz, K], f32, tag=f"Pt{xj}_{b}",
                                name=f"Pt{xj}_{b}")
                    nc.vector.tensor_scalar_mul(t[:], tall[:, b * K:(b + 1) * K],
                                                mask_sb[0:xsz, b:b + 1])
                    nc.vector.scalar_tensor_tensor(
                        out=t[:], in0=tall[:, (b + 1) * K:(b + 2) * K],
                        scalar=mask_sb[0:xsz, b + 1:b + 2], in1=t[:],
                        op0=mybir.AluOpType.mult, op1=mybir.AluOpType.add,
                    )
                    pairs.append(t)
                nc.vector.tensor_add(pairs[0][:], pairs[0][:], pairs[1][:])
                nc.vector.tensor_add(pairs[2][:], pairs[2][:], pairs[3][:])
                pfx = qp.tile([xsz, K], f32, tag=f"P{xj}", name=f"P{xj}")
                nc.vector.tensor_add(pfx[:], pairs[0][:], pairs[2][:])
                P.append(pfx)

            # ---- prefix add + store S chunks + chunked AllGather -----------
            XA = W + 1                       # 193 shipped x rows (x<=192)
            ag_ins = [dramp.tile([XA, CH[c] * K], f32, tag=f"ag_in{c}",
                                 name=f"ag_in{c}") for c in range(nch)]
            ag_outs = [dramp.tile([NCORES * XA * CH[c], K], f32,
                                  tag=f"ag_out{c}", name=f"ag_out{c}",
                                  addr_space="Shared") for c in range(nch)]
            for c in range(nch):
                for xj, (xoff, xsz) in enumerate(xch):
                    ssz = min(xsz, XA - xoff)
                    qv = Qc[xj][:, HO[c] * K:(HO[c] + CH[c]) * K]
                    pb = P[xj][:].unsqueeze(1).broadcast_to([xsz, CH[c], K])
                    nc.vector.tensor_add(
                        qv.rearrange("x (h k) -> x h k", k=K),
                        qv.rearrange("x (h k) -> x h k", k=K), pb)
                    nc.sync.dma_start(ag_ins[c][xoff:xoff + ssz, :],
                                      Qc[xj][0:ssz, HO[c] * K:(HO[c] + CH[c]) * K])
                nc.gpsimd.collective_compute(
                    "AllGather", mybir.AluOpType.bypass, replica_groups=RG,
                    ins=[ag_ins[c][:].opt()], outs=[ag_outs[c][:].opt()],
                )

            # ---- corner gathers + combine ----------------------------------
            # plan = (NB_SB, NB_XB): SB batches gather all 4 corners from the
            # LOCAL prefixed block (ag_in) — they run during the AllGather.
            # XB batches gather y1-corners (j0/j2) locally (early) and
            # y0-corners (j1/j3) from ag_out (after the AllGather).
            NB_SB, NB_XB = plan
            loc_rows = ag_ins[0].rearrange("x (h k) -> (x h) k", k=K)
            pv = pred.rearrange("(m p) k -> p m k", p=128)

            def gather(tile_ap, src, col):
                nc.gpsimd.indirect_dma_start(
                    out=tile_ap, out_offset=None, in_=src,
                    in_offset=bass.IndirectOffsetOnAxis(
                        ap=idx_sb[:, col:col + 1], axis=0))

            def combine(g, m):
                nc.vector.tensor_sub(g[0][:], g[0][:], g[1][:])
                nc.vector.tensor_sub(g[2][:], g[2][:], g[3][:])
                nc.vector.tensor_sub(g[0][:], g[0][:], g[2][:])
                nc.vector.tensor_scalar_mul(g[1][:], g[0][:],
                                            iar_sb[:, m:m + 1])
                nc.sync.dma_start(pv[:, m, :], g[1][:])

            for m in range(NB_SB):              # all-local batches, early
                g = []
                for ci in range(4):
                    gt = gatp.tile([128, K], f32, tag=f"gt{ci}", name=f"gt{ci}")
                    gather(gt[:], loc_rows, ci * NB + m)
                    g.append(gt)
                combine(g, m)
            xb_tiles = {}
            for m in range(NB_SB, NB):          # y1-corners, early (local)
                for ci in (0, 2):
                    gt = xbp.tile([128, K], f32, tag=f"xb{ci}_{m}",
                                  name=f"xb{ci}_{m}")
                    gather(gt[:], loc_rows, ci * NB + m)
                    xb_tiles[(m, ci)] = gt
            for m in range(NB_SB, NB):          # y0-corners, after AllGather
                g1 = gatp.tile([128, K], f32, tag="gt1", name="gt1")
                gather(g1[:], ag_outs[0][:], 1 * NB + m)
                g3 = gatp.tile([128, K], f32, tag="gt3", name="gt3")
                gather(g3[:], ag_outs[0][:], 3 * NB + m)
                combine([xb_tiles[(m, 0)], g1, xb_tiles[(m, 2)], g3], m)

    nc.compile()
    return nc


def _get_nc(CH, plan):
    key = (tuple(CH), tuple(plan))
    if key not in _NC_CACHE:
        _NC_CACHE[key] = _build_nc(list(CH), list(plan))
    return _NC_CACHE[key]


def _prepare(feature_map, scale, anchors, fc_w, anchor_num, CH):
    """Host-side prep: swizzle fm, tri, per-core grouped corner plan."""
    import ml_dtypes
    bf = ml_dtypes.bfloat16

    N = int(anchor_num)
    assert N == N_ANCH, N
    anchors = np.asarray(anchors, dtype=np.float32)[:N]
    x0, x1, y0, y1 = _box_indices_np(anchors, np.asarray(scale, np.float32), H, W)
    area = np.maximum((y1 - y0) * (x1 - x0), 1).astype(np.float32)
    inv_area = (np.float32(1.0) / area).astype(np.float32)

    assert len(CH) == 1 and CH[0] == HSH
    XA = W + 1
    blk1, hh1 = (y1 - 1) // HSH, (y1 - 1) % HSH
    y0c = np.maximum(y0, 1)
    blk0, hh0 = (y0c - 1) // HSH, (y0c - 1) % HSH

    def lrid(x, hh, zero):
        # row id within the local prefixed block ag_in, viewed [XA*HSH, K]
        return np.where(zero, 0, x * HSH + hh).astype(np.int32)

    def grid(blk, x, hh, zero):
        # row id within the allgathered integral image
        return np.where(zero, 0,
                        blk * (XA * HSH) + x * HSH + hh).astype(np.int32)

    f_ = np.zeros_like(x1, dtype=bool)
    r0 = lrid(x1, hh1, f_)                              # (x1,y1) local
    r2 = lrid(x0, hh1, x0 == 0)                         # (x0,y1) local
    # SB anchors: y0 in same block (or y0==0 -> zero row): j1/j3 local too
    sb = (y0 == 0) | (blk0 == blk1)
    r1 = np.where(sb, lrid(x1, hh0, y0 == 0),
                  grid(blk0, x1, hh0, f_)).astype(np.int32)
    r3 = np.where(sb, lrid(x0, hh0, (x0 == 0) | (y0 == 0)),
                  grid(blk0, x0, hh0, x0 == 0)).astype(np.int32)
    corners = np.stack([r0, r1, r2, r3])                # [4, N]

    fcwT = np.ascontiguousarray(fc_w.T.astype(bf))                 # [C, K]
    fcw_in = np.ascontiguousarray(
        fcwT.reshape(CCH, 128, K).transpose(1, 0, 2))
    tri = np.zeros((W, XP), dtype=np.float32)
    for x in range(1, W + 1):
        tri[0:x, x] = 1.0
    trib = tri.astype(bf)

    maskf = np.zeros((NCORES, 128, NCORES), dtype=np.float32)
    for i in range(NCORES):
        maskf[i, :, :i] = 1.0

    # assign every anchor to the core owning its y1 block; SB (fully local)
    # anchors first, then XB.  One SPMD plan: pad both sections to the max
    # batch count over cores.
    per_core = []
    for i in range(NCORES):
        mine = np.nonzero(blk1 == i)[0]
        sb_i = mine[sb[mine]]
        xb_i = mine[~sb[mine]]
        per_core.append((sb_i, xb_i))
    NB_SB = max((len(s) + 127) // 128 for s, _ in per_core)
    NB_XB = max((len(x) + 127) // 128 for _, x in per_core)
    plan = (NB_SB, NB_XB)
    NB = NB_SB + NB_XB

    in_maps = []
    slot_maps = []
    for i in range(NCORES):
        sb_i, xb_i = per_core[i]
        cid = np.zeros((4, NB * 128), dtype=np.int32)
        ia = np.zeros(NB * 128, dtype=np.float32)
        slots = np.full(NB * 128, -1, dtype=np.int64)
        for idxs, off in ((sb_i, 0), (xb_i, NB_SB * 128)):
            gsl = slice(off, off + len(idxs))
            cid[:, gsl] = corners[:, idxs]
            ia[gsl] = inv_area[idxs]
            slots[gsl] = idxs
        # reshape to [4, NB, 128] -> [4, 128, NB] slot (m, p) = m*128+p
        cid = cid.reshape(4, NB, 128).transpose(0, 2, 1)
        ia2 = ia.reshape(NB, 128).T
        slot_maps.append(slots)

        fm_i = feature_map[:, i * HSH:(i + 1) * HSH, :].astype(bf)
        fm_i = fm_i.reshape(CCH, 128, HQ, HR, W).transpose(1, 2, 0, 3, 4)
        in_maps.append({
            "fm": np.ascontiguousarray(fm_i),
            "fcw": fcw_in,
            "trib": trib,
            "cidx": np.ascontiguousarray(cid),
            "iar": np.ascontiguousarray(ia2),
            "mask": np.ascontiguousarray(maskf[i]),
        })
    return in_maps, slot_maps, plan


def kernel(**inputs):
    global LAST_RESULTS
    feature_map = np.asarray(inputs["feature_map"], dtype=np.float32)
    scale = np.asarray(inputs["scale"], dtype=np.float32)
    anchors = np.asarray(inputs["anchors"], dtype=np.float32)
    fc_w = np.asarray(inputs["fc_w"], dtype=np.float32)
    fc_b = np.asarray(inputs["fc_b"], dtype=np.float32)
    anchor_num = int(np.asarray(inputs["anchor_num"]))

    import time
    CH = _chunk_list()
    t0 = time.time()
    in_maps, slot_maps, plan = _prepare(feature_map, scale, anchors, fc_w,
                                        anchor_num, CH)
    print(f"[kernel] host prep {time.time() - t0:.1f}s NB={len(plan)}", flush=True)
    t0 = time.time()
    nc = _get_nc(CH, plan)
    print(f"[kernel] bass build+schedule {time.time() - t0:.1f}s", flush=True)

    from concourse.bass_utils import run_bass_kernel_spmd
    trace = bool(int(os.environ.get("NMS_TRACE", "0")))
    t0 = time.time()
    res = run_bass_kernel_spmd(nc, in_maps, core_ids=list(range(NCORES)),
                               trace=trace)
    print(f"[kernel] compile+run {time.time() - t0:.1f}s", flush=True)
    LAST_RESULTS = res
    pred = np.empty((N_ANCH, K), dtype=np.float32)
    for i in range(NCORES):
        block = res.results[i]["pred"]          # [NB*128, K] grouped order
        slots = slot_maps[i]                    # global anchor ids
        valid = slots >= 0
        pred[slots[valid]] = block[valid]
    return (pred + fc_b[None, :].astype(np.float32)).astype(np.float32)


# revision 28
# speedup vs baseline: 1.1379x; 1.1379x over previous
"""Bass/Trainium2 kernel for nn_FC_Classifier (box-pooled FC classifier).

Math: pred[n,k] = (1/area_n) * sum_{(h,w) in box_n} (fc_w @ feature_map)[k,h,w] + fc_b[k]

Strategy (8 cores, one chip), v3:
  * Shard image rows h across cores (24 rows/core).  Phase 1 contracts
    channels (2048 -> 150) with matmuls (host-swizzled fm, contiguous DMA).
  * W-cumsum fused per image row via triangular matmul (bf16 tri, f32 PSUM).
  * H-cumsum fused into the PSUM->SBUF copies as a running add chain.
  * Tiny AllGather of per-block column totals + mask-weighted prefix sum.
  * Integral image AllGather split into h-chunks (separate Shared tensors);
    anchors are grouped so every corner-gather instruction reads exactly one
    chunk tensor, so gathers overlap the later AllGather chunks' wire time.
  * 4-corner indirect-DMA gathers, combine on DVE, scale by 1/area; bias on
    host.

Self-contained: only numpy + the concourse (Bass) runtime are imported.
"""

import os
import numpy as np

DS = 8.0
NCORES = 8
C, H, W, K, N_ANCH = 2048, 192, 192, 150, 16384
HSH = H // NCORES              # 24 image rows per core
XP = 200                       # x range of S (0..192 used), padded to 8*25
CCH = C // 128                 # 16 channel chunks
HQ = 12                        # fm DMA chunks (2 rows each)
HR = HSH // HQ

LAST_RESULTS = None  # BassKernelResults of the most recent run (for test.py)

_NC_CACHE = {}


def _chunks(total, size):
    return [(o, min(size, total - o)) for o in range(0, total, size)]


def _chunk_list():
    s = os.environ.get("NMS_AG_CHUNKS", "24")
    ch = [int(x) for x in s.split(",") if x]
    assert sum(ch) == HSH, ch
    return ch


def _box_indices_np(anchors, scale, h, w):
    # exact replica of reference._box_indices in numpy f32
    a = anchors.astype(np.float32) / np.float32(DS)
    x0 = (a[:, 0] * scale[1]).astype(np.int32)
    x1 = (a[:, 1] * scale[1]).astype(np.int32)
    y0 = (a[:, 2] * scale[0]).astype(np.int32)
    y1 = (a[:, 3] * scale[0]).astype(np.int32)
    eqy = y0 == y1
    y0, y1 = (
        np.where(eqy & (y0 != 0), y0 - 1, y0),
        np.where(eqy & (y0 == 0), y1 + 1, y1),
    )
    eqx = x0 == x1
    x0, x1 = (
        np.where(eqx & (x0 != 0), x0 - 1, x0),
        np.where(eqx & (x0 == 0), x1 + 1, x1),
    )
    y0, y1 = np.clip(y0, 0, h), np.clip(y1, 0, h)
    x0, x1 = np.clip(x0, 0, w), np.clip(x1, 0, w)
    return x0, x1, y0, y1


def _build_nc(CH, plan):
    """Build the SPMD Bass program (identical on all 8 cores).

    CH: h-chunk sizes for the big AllGather (sum = 24).
    plan: per gather batch, (a, b) = chunk tensor ids for the y1-corners
          (j0/j2) and y0-corners (j1/j3).
    """
    from concourse import bacc, mybir, tile
    import concourse.bass as bass

    f32 = mybir.dt.float32
    bf16 = mybir.dt.bfloat16
    i32 = mybir.dt.int32

    wch = _chunks(W, 128)          # [(0,128),(128,64)]   w partition chunks
    xch = _chunks(XP, 128)         # [(0,128),(128,72)]   x partition chunks
    HO = np.concatenate([[0], np.cumsum(CH)]).astype(int)
    nch = len(CH)
    NB = sum(plan)

    nc = bacc.Bacc("TRN2", target_bir_lowering=False, debug=False,
                   num_devices=NCORES)
    # host-swizzled fm: [p, hq, cc, hr, w] so each h-chunk DMA is contiguous
    fm = nc.dram_tensor("fm", [128, HQ, CCH, HR, W], bf16, kind="ExternalInput").ap()
    fcw = nc.dram_tensor("fcw", [128, CCH, K], bf16, kind="ExternalInput").ap()
    trib = nc.dram_tensor("trib", [W, XP], bf16, kind="ExternalInput").ap()
    cidx = nc.dram_tensor("cidx", [4, 128, NB], i32, kind="ExternalInput").ap()
    iar = nc.dram_tensor("iar", [128, NB], f32, kind="ExternalInput").ap()
    mask = nc.dram_tensor("mask", [128, NCORES], f32, kind="ExternalInput").ap()
    pred = nc.dram_tensor("pred", [128 * NB, K], f32, kind="ExternalOutput").ap()

    RG = [list(range(NCORES))]
    NF = HSH * K

    with tile.TileContext(nc) as tc:
        with (
            tc.tile_pool(name="constp", bufs=1) as constp,
            tc.tile_pool(name="fmp", bufs=3) as fmp,
            tc.tile_pool(name="gp", bufs=3) as gp,
            tc.tile_pool(name="qp", bufs=1) as qp,
            tc.tile_pool(name="psp", bufs=2, space="PSUM") as psp,
            tc.tile_pool(name="gatp", bufs=12) as gatp,
            tc.tile_pool(name="xbp", bufs=1) as xbp,
            tc.tile_pool(name="dramp", bufs=1, space="DRAM") as dramp,
        ):
            # ---- constants -------------------------------------------------
            fcw_sb = constp.tile([128, CCH * K], bf16, tag="fcw", name="fcw_sb")
            nc.sync.dma_start(fcw_sb[:], fcw.rearrange("p cc k -> p (cc k)"))

            tri_w = []                       # [wsz, XP] per w-chunk
            for j, (off, sz) in enumerate(wch):
                t = constp.tile([sz, XP], bf16, tag=f"tri_w{j}", name=f"tri_w{j}")
                nc.sync.dma_start(t[:], trib[off:off + sz, :])
                tri_w.append(t)

            idx_sb = constp.tile([128, 4 * NB], i32, tag="idx", name="idx_sb")
            nc.sync.dma_start(idx_sb[:], cidx.rearrange("c p m -> p c m"))
            iar_sb = constp.tile([128, NB], f32, tag="iar", name="iar_sb")
            nc.sync.dma_start(iar_sb[:], iar[:, :])
            mask_sb = constp.tile([128, NCORES], f32, tag="mask", name="mask_sb")
            nc.sync.dma_start(mask_sb[:], mask[:, :])

            # persistent Q-cumsum buffers [x, (h k)]
            Qc = [qp.tile([sz, NF], f32, tag=f"Qc{j}", name=f"Qc{j}")
                  for j, (off, sz) in enumerate(xch)]

            # warm up the collective path early so the first real collective
            # doesn't pay the ncfw entry latency
            warm_in = dramp.tile([128, 4], f32, tag="warm_in", name="warm_in")
            warm_out = dramp.tile([128, 4], f32, tag="warm_out",
                                  name="warm_out", addr_space="Shared")
            wsb = constp.tile([128, 4], f32, tag="warm_sb", name="warm_sb")
            nc.gpsimd.memset(wsb[:], 0.0)
            nc.sync.dma_start(warm_in[:, :], wsb[:])
            nc.gpsimd.collective_compute(
                "AllReduce", mybir.AluOpType.add, replica_groups=RG,
                ins=[warm_in[:].opt()], outs=[warm_out[:].opt()],
            )

            # ---- phase 1: projection + W-cumsum + fused H-cumsum -----------
            for hq in range(HQ):
                fmh = fmp.tile([128, CCH * HR * W], bf16, tag="fmh", name="fmh")
                nc.sync.dma_start(fmh[:], fm.rearrange("p hq cc hr w -> p hq (cc hr w)")[:, hq])
                for hr in range(HR):
                    h = hq * HR + hr
                    gts = []
                    for wj, (woff, wsz) in enumerate(wch):
                        ps = psp.tile([wsz, K], f32, tag=f"pp{wj}", name="ps1")
                        for cc in range(CCH):
                            o = cc * (HR * W) + hr * W + woff
                            nc.tensor.matmul(
                                ps[:],
                                lhsT=fmh[:, o: o + wsz],
                                rhs=fcw_sb[:, cc * K:(cc + 1) * K],
                                start=(cc == 0), stop=(cc == CCH - 1),
                            )
                        gt = gp.tile([wsz, K], bf16, tag=f"g{wj}", name=f"g{wj}")
                        nc.vector.tensor_copy(gt[:], ps[:])
                        gts.append(gt)
                    for xj, (xoff, xsz) in enumerate(xch):
                        qs = psp.tile([xsz, K], f32, tag=f"wp{xj}", name="ps2")
                        for wj in range(len(wch)):
                            nc.tensor.matmul(
                                qs[:],
                                lhsT=tri_w[wj][:, xoff:xoff + xsz],
                                rhs=gts[wj][:],
                                start=(wj == 0), stop=(wj == len(wch) - 1),
                            )
                        if h == 0:
                            nc.vector.tensor_copy(Qc[xj][:, 0:K], qs[:])
                        else:
                            nc.vector.tensor_add(
                                Qc[xj][:, h * K:(h + 1) * K], qs[:],
                                Qc[xj][:, (h - 1) * K:h * K])

            # ---- block totals exchange (replicated AllToAll = AllGather) ---
            ag2_in = dramp.tile([NCORES * XP, K], f32, tag="ag2_in",
                                name="ag2_in")
            ag2_out = dramp.tile([NCORES * XP, K], f32, tag="ag2_out",
                                 name="ag2_out")
            a2iv = ag2_in.rearrange("(b x) k -> x b k", b=NCORES)
            for xj, (xoff, xsz) in enumerate(xch):
                src = (Qc[xj][:, (HSH - 1) * K:HSH * K]
                       .unsqueeze(1).broadcast_to([xsz, NCORES, K]))
                nc.sync.dma_start(a2iv[xoff:xoff + xsz], src)
            nc.gpsimd.collective_compute(
                "AllToAll", mybir.AluOpType.bypass, replica_groups=RG,
                ins=[ag2_in[:].opt()], outs=[ag2_out[:].opt()],
            )
            a2v = ag2_out.rearrange("(b x) k -> x b k", b=NCORES)
            P = []
            for xj, (xoff, xsz) in enumerate(xch):
                tall = qp.tile([xsz, NCORES * K], f32, tag=f"tall{xj}",
                               name=f"tall{xj}")
                nc.sync.dma_start(
                    tall[:].rearrange("x (b k) -> x b k", b=NCORES),
                    a2v[xoff:xoff + xsz])
                # tree-structured masked sum: 4 fused ops + 3 adds
                pairs = []
                for b in range(0, NCORES, 2):
                    t = qp.tile([xsz, K], f32, tag=f"Pt{xj}_{b}",
                                name=f"Pt{xj}_{b}")
                    nc.vector.tensor_scalar_mul(t[:], tall[:, b * K:(b + 1) * K],
                                                mask_sb[0:xsz, b:b + 1])
                    nc.vector.scalar_tensor_tensor(
                        out=t[:], in0=tall[:, (b + 1) * K:(b + 2) * K],
                        scalar=mask_sb[0:xsz, b + 1:b + 2], in1=t[:],
                        op0=mybir.AluOpType.mult, op1=mybir.AluOpType.add,
                    )
                    pairs.append(t)
                nc.vector.tensor_add(pairs[0][:], pairs[0][:], pairs[1][:])
                nc.vector.tensor_add(pairs[2][:], pairs[2][:], pairs[3][:])
                pfx = qp.tile([xsz, K], f32, tag=f"P{xj}", name=f"P{xj}")
                nc.vector.tensor_add(pfx[:], pairs[0][:], pairs[2][:])
                P.append(pfx)

            # ---- prefix add + store S chunks + chunked AllGather -----------
            XA = W + 1                       # 193 shipped x rows (x<=192)
            ag_ins = [dramp.tile([XA, CH[c] * K], f32, tag=f"ag_in{c}",
                                 name=f"ag_in{c}") for c in range(nch)]
            ag_outs = [dramp.tile([NCORES * XA * CH[c], K], f32,
                                  tag=f"ag_out{c}", name=f"ag_out{c}",
                                  addr_space="Shared") for c in range(nch)]
            for c in range(nch):
                for xj, (xoff, xsz) in enumerate(xch):
                    ssz = min(xsz, XA - xoff)
                    qv = Qc[xj][:, HO[c] * K:(HO[c] + CH[c]) * K]
                    pb = P[xj][:].unsqueeze(1).broadcast_to([xsz, CH[c], K])
                    nc.vector.tensor_add(
                        qv.rearrange("x (h k) -> x h k", k=K),
                        qv.rearrange("x (h k) -> x h k", k=K), pb)
                    nc.sync.dma_start(ag_ins[c][xoff:xoff + ssz, :],
                                      Qc[xj][0:ssz, HO[c] * K:(HO[c] + CH[c]) * K])
                nc.gpsimd.collective_compute(
                    "AllGather", mybir.AluOpType.bypass, replica_groups=RG,
                    ins=[ag_ins[c][:].opt()], outs=[ag_outs[c][:].opt()],
                )

            # ---- corner gathers + combine ----------------------------------
            # plan = (NB_SB, NB_XB): SB batches gather all 4 corners from the
            # LOCAL prefixed block (ag_in) — they run during the AllGather.
            # XB batches gather y1-corners (j0/j2) locally (early) and
            # y0-corners (j1/j3) from ag_out (after the AllGather).
            NB_SB, NB_L1, NB_L0, NB_G = plan
            loc_rows = ag_ins[0].rearrange("x (h k) -> (x h) k", k=K)
            pv = pred.rearrange("(m p) k -> p m k", p=128)

            def gather(tile_ap, src, col):
                nc.gpsimd.indirect_dma_start(
                    out=tile_ap, out_offset=None, in_=src,
                    in_offset=bass.IndirectOffsetOnAxis(
                        ap=idx_sb[:, col:col + 1], axis=0))

            def combine(g, m):
                nc.vector.tensor_sub(g[0][:], g[0][:], g[1][:])
                nc.vector.tensor_sub(g[2][:], g[2][:], g[3][:])
                nc.vector.tensor_sub(g[0][:], g[0][:], g[2][:])
                nc.vector.tensor_scalar_mul(g[1][:], g[0][:],
                                            iar_sb[:, m:m + 1])
                nc.sync.dma_start(pv[:, m, :], g[1][:])

            for m in range(NB_SB):              # all-local batches, early
                g = []
                for ci in range(4):
                    gt = gatp.tile([128, K], f32, tag=f"gt{ci}", name=f"gt{ci}")
                    gather(gt[:], loc_rows, ci * NB + m)
                    g.append(gt)
                combine(g, m)
            # L1: y1-corners (j0/j2) local early; L0: y0-corners (j1/j3)
            early = {}
            for m in range(NB_SB, NB_SB + NB_L1):
                for ci in (0, 2):
                    gt = xbp.tile([128, K], f32, tag=f"xb{ci}_{m}",
                                  name=f"xb{ci}_{m}")
                    gather(gt[:], loc_rows, ci * NB + m)
                    early[(m, ci)] = gt
            for m in range(NB_SB + NB_L1, NB_SB + NB_L1 + NB_L0):
                for ci in (1, 3):
                    gt = xbp.tile([128, K], f32, tag=f"xb{ci}_{m}",
                                  name=f"xb{ci}_{m}")
                    gather(gt[:], loc_rows, ci * NB + m)
                    early[(m, ci)] = gt
            # late gathers from the allgathered image + combines
            for m in range(NB_SB, NB):
                if m < NB_SB + NB_L1:
                    late_ci = (1, 3)
                elif m < NB_SB + NB_L1 + NB_L0:
                    late_ci = (0, 2)
                else:
                    late_ci = (0, 1, 2, 3)
                g = [None] * 4
                for ci in late_ci:
                    gt = gatp.tile([128, K], f32, tag=f"gl{ci}", name=f"gl{ci}")
                    gather(gt[:], ag_outs[0][:], ci * NB + m)
                    g[ci] = gt
                for ci in (0, 1, 2, 3):
                    if g[ci] is None:
                        g[ci] = early[(m, ci)]
                combine(g, m)

    nc.compile()
    return nc


def _get_nc(CH, plan):
    key = (tuple(CH), tuple(plan))
    if key not in _NC_CACHE:
        _NC_CACHE[key] = _build_nc(list(CH), list(plan))
    return _NC_CACHE[key]


def _prepare(feature_map, scale, anchors, fc_w, anchor_num, CH):
    """Host-side prep: swizzle fm, tri, per-core grouped corner plan."""
    import ml_dtypes
    bf = ml_dtypes.bfloat16

    N = int(anchor_num)
    assert N == N_ANCH, N
    anchors = np.asarray(anchors, dtype=np.float32)[:N]
    x0, x1, y0, y1 = _box_indices_np(anchors, np.asarray(scale, np.float32), H, W)
    area = np.maximum((y1 - y0) * (x1 - x0), 1).astype(np.float32)
    inv_area = (np.float32(1.0) / area).astype(np.float32)

    assert len(CH) == 1 and CH[0] == HSH
    XA = W + 1
    blk1, hh1 = (y1 - 1) // HSH, (y1 - 1) % HSH
    y0c = np.maximum(y0, 1)
    blk0, hh0 = (y0c - 1) // HSH, (y0c - 1) % HSH

    def lrid(x, hh, zero):
        # row id within the local prefixed block ag_in, viewed [XA*HSH, K]
        return np.where(zero, 0, x * HSH + hh).astype(np.int32)

    def grid(blk, x, hh, zero):
        # row id within the allgathered integral image
        return np.where(zero, 0,
                        blk * (XA * HSH) + x * HSH + hh).astype(np.int32)

    f_ = np.zeros_like(x1, dtype=bool)
    # SB anchors: y0 in same block (or y0==0 -> zero row): all 4 local
    sb = (y0 == 0) | (blk0 == blk1)
    # capacity-capped greedy assignment; spill goes to an all-global section
    N_ = len(x1)
    cap = N_ // NCORES
    owner = np.full(N_, -1, dtype=np.int64)
    load = np.zeros(NCORES, dtype=np.int64)
    pool = []
    for a in np.nonzero(sb)[0]:
        b = blk1[a]
        if load[b] < cap:
            owner[a] = b
            load[b] += 1
        else:
            pool.append(a)
    for a in np.nonzero(~sb)[0]:
        b0_, b1_ = blk0[a], blk1[a]
        cands = [b for b in (b0_, b1_) if load[b] < cap]
        if cands:
            b = min(cands, key=lambda c: load[c])
            owner[a] = b
            load[b] += 1
        else:
            pool.append(a)
    for a in pool:
        b = int(np.argmin(load))
        owner[a] = b
        load[b] += 1
    is_sb = sb & (owner == blk1)
    is_l1 = (~sb) & (owner == blk1)
    is_l0 = (~sb) & (owner == blk0)
    is_g = ~(is_sb | is_l1 | is_l0)
    use_l1 = is_sb | is_l1             # y1-corners local on owner
    use_l0 = is_sb | is_l0             # y0-corners local on owner
    r0 = np.where(use_l1, lrid(x1, hh1, f_), grid(blk1, x1, hh1, f_))
    r2 = np.where(use_l1, lrid(x0, hh1, x0 == 0), grid(blk1, x0, hh1, x0 == 0))
    z1 = y0 == 0
    z3 = (x0 == 0) | (y0 == 0)
    r1 = np.where(use_l0, lrid(x1, hh0, z1), grid(blk0, x1, hh0, z1))
    r3 = np.where(use_l0, lrid(x0, hh0, z3), grid(blk0, x0, hh0, z3))
    corners = np.stack([r0, r1, r2, r3]).astype(np.int32)   # [4, N]

    fcwT = np.ascontiguousarray(fc_w.T.astype(bf))                 # [C, K]
    fcw_in = np.ascontiguousarray(
        fcwT.reshape(CCH, 128, K).transpose(1, 0, 2))
    tri = np.zeros((W, XP), dtype=np.float32)
    for x in range(1, W + 1):
        tri[0:x, x] = 1.0
    trib = tri.astype(bf)

    maskf = np.zeros((NCORES, 128, NCORES), dtype=np.float32)
    for i in range(NCORES):
        maskf[i, :, :i] = 1.0

    # per-core sections: SB / L1 / L0 / G, one SPMD plan padded to maxes
    per_core = []
    for i in range(NCORES):
        mine = owner == i
        per_core.append((np.nonzero(mine & is_sb)[0],
                         np.nonzero(mine & is_l1)[0],
                         np.nonzero(mine & is_l0)[0],
                         np.nonzero(mine & is_g)[0]))
    NB_SB = max((len(t[0]) + 127) // 128 for t in per_core)
    NB_L1 = max((len(t[1]) + 127) // 128 for t in per_core)
    NB_L0 = max((len(t[2]) + 127) // 128 for t in per_core)
    NB_G = max((len(t[3]) + 127) // 128 for t in per_core)
    plan = (NB_SB, NB_L1, NB_L0, NB_G)
    NB = sum(plan)

    in_maps = []
    slot_maps = []
    for i in range(NCORES):
        sec = per_core[i]
        cid = np.zeros((4, NB * 128), dtype=np.int32)
        ia = np.zeros(NB * 128, dtype=np.float32)
        slots = np.full(NB * 128, -1, dtype=np.int64)
        off = 0
        for si, idxs in enumerate(sec):
            gsl = slice(off, off + len(idxs))
            cid[:, gsl] = corners[:, idxs]
            ia[gsl] = inv_area[idxs]
            slots[gsl] = idxs
            off += plan[si] * 128
        # reshape to [4, NB, 128] -> [4, 128, NB] slot (m, p) = m*128+p
        cid = cid.reshape(4, NB, 128).transpose(0, 2, 1)
        ia2 = ia.reshape(NB, 128).T
        slot_maps.append(slots)

        fm_i = feature_map[:, i * HSH:(i + 1) * HSH, :].astype(bf)
        fm_i = fm_i.reshape(CCH, 128, HQ, HR, W).transpose(1, 2, 0, 3, 4)
        in_maps.append({
            "fm": np.ascontiguousarray(fm_i),
            "fcw": fcw_in,
            "trib": trib,
            "cidx": np.ascontiguousarray(cid),
            "iar": np.ascontiguousarray(ia2),
            "mask": np.ascontiguousarray(maskf[i]),
        })
    return in_maps, slot_maps, plan


def kernel(**inputs):
    global LAST_RESULTS
    feature_map = np.asarray(inputs["feature_map"], dtype=np.float32)
    scale = np.asarray(inputs["scale"], dtype=np.float32)
    anchors = np.asarray(inputs["anchors"], dtype=np.float32)
    fc_w = np.asarray(inputs["fc_w"], dtype=np.float32)
    fc_b = np.asarray(inputs["fc_b"], dtype=np.float32)
    anchor_num = int(np.asarray(inputs["anchor_num"]))

    import time
    CH = _chunk_list()
    t0 = time.time()
    in_maps, slot_maps, plan = _prepare(feature_map, scale, anchors, fc_w,
                                        anchor_num, CH)
    print(f"[kernel] host prep {time.time() - t0:.1f}s NB={len(plan)}", flush=True)
    t0 = time.time()
    nc = _get_nc(CH, plan)
    print(f"[kernel] bass build+schedule {time.time() - t0:.1f}s", flush=True)

    from concourse.bass_utils import run_bass_kernel_spmd
    trace = bool(int(os.environ.get("NMS_TRACE", "0")))
    t0 = time.time()
    res = run_bass_kernel_spmd(nc, in_maps, core_ids=list(range(NCORES)),
                               trace=trace)
    print(f"[kernel] compile+run {time.time() - t0:.1f}s", flush=True)
    LAST_RESULTS = res
    pred = np.empty((N_ANCH, K), dtype=np.float32)
    for i in range(NCORES):
        block = res.results[i]["pred"]          # [NB*128, K] grouped order
        slots = slot_maps[i]                    # global anchor ids
        valid = slots >= 0
        pred[slots[valid]] = block[valid]
    return (pred + fc_b[None, :].astype(np.float32)).astype(np.float32)


# revision 29
# speedup vs baseline: 1.2618x; 1.1089x over previous
"""Bass/Trainium2 kernel for nn_FC_Classifier (box-pooled FC classifier).

Math: pred[n,k] = (1/area_n) * sum_{(h,w) in box_n} (fc_w @ feature_map)[k,h,w] + fc_b[k]

Strategy (8 cores, one chip), v3:
  * Shard image rows h across cores (24 rows/core).  Phase 1 contracts
    channels (2048 -> 150) with matmuls (host-swizzled fm, contiguous DMA).
  * W-cumsum fused per image row via triangular matmul (bf16 tri, f32 PSUM).
  * H-cumsum fused into the PSUM->SBUF copies as a running add chain.
  * Tiny AllGather of per-block column totals + mask-weighted prefix sum.
  * Integral image AllGather split into h-chunks (separate Shared tensors);
    anchors are grouped so every corner-gather instruction reads exactly one
    chunk tensor, so gathers overlap the later AllGather chunks' wire time.
  * 4-corner indirect-DMA gathers, combine on DVE, scale by 1/area; bias on
    host.

Self-contained: only numpy + the concourse (Bass) runtime are imported.
"""

import os
import numpy as np

DS = 8.0
NCORES = 8
C, H, W, K, N_ANCH = 2048, 192, 192, 150, 16384
HSH = H // NCORES              # 24 image rows per core
XP = 200                       # x range of S (0..192 used), padded to 8*25
CCH = C // 128                 # 16 channel chunks
HQ = 12                        # fm DMA chunks (2 rows each)
HR = HSH // HQ

LAST_RESULTS = None  # BassKernelResults of the most recent run (for test.py)

_NC_CACHE = {}


def _chunks(total, size):
    return [(o, min(size, total - o)) for o in range(0, total, size)]


def _chunk_list():
    s = os.environ.get("NMS_AG_CHUNKS", "24")
    ch = [int(x) for x in s.split(",") if x]
    assert sum(ch) == HSH, ch
    return ch


def _box_indices_np(anchors, scale, h, w):
    # exact replica of reference._box_indices in numpy f32
    a = anchors.astype(np.float32) / np.float32(DS)
    x0 = (a[:, 0] * scale[1]).astype(np.int32)
    x1 = (a[:, 1] * scale[1]).astype(np.int32)
    y0 = (a[:, 2] * scale[0]).astype(np.int32)
    y1 = (a[:, 3] * scale[0]).astype(np.int32)
    eqy = y0 == y1
    y0, y1 = (
        np.where(eqy & (y0 != 0), y0 - 1, y0),
        np.where(eqy & (y0 == 0), y1 + 1, y1),
    )
    eqx = x0 == x1
    x0, x1 = (
        np.where(eqx & (x0 != 0), x0 - 1, x0),
        np.where(eqx & (x0 == 0), x1 + 1, x1),
    )
    y0, y1 = np.clip(y0, 0, h), np.clip(y1, 0, h)
    x0, x1 = np.clip(x0, 0, w), np.clip(x1, 0, w)
    return x0, x1, y0, y1


def _build_nc(CH, plan):
    """Build the SPMD Bass program (identical on all 8 cores).

    CH: h-chunk sizes for the big AllGather (sum = 24).
    plan: per gather batch, (a, b) = chunk tensor ids for the y1-corners
          (j0/j2) and y0-corners (j1/j3).
    """
    from concourse import bacc, mybir, tile
    import concourse.bass as bass

    f32 = mybir.dt.float32
    bf16 = mybir.dt.bfloat16
    i32 = mybir.dt.int32

    wch = _chunks(W, 128)          # [(0,128),(128,64)]   w partition chunks
    xch = _chunks(XP, 128)         # [(0,128),(128,72)]   x partition chunks
    HO = np.concatenate([[0], np.cumsum(CH)]).astype(int)
    nch = len(CH)
    NB = sum(plan)

    nc = bacc.Bacc("TRN2", target_bir_lowering=False, debug=False,
                   num_devices=NCORES)
    # host-swizzled fm: [p, hq, cc, hr, w] so each h-chunk DMA is contiguous
    fm = nc.dram_tensor("fm", [128, HQ, CCH, HR, W], bf16, kind="ExternalInput").ap()
    fcw = nc.dram_tensor("fcw", [128, CCH, K], bf16, kind="ExternalInput").ap()
    trib = nc.dram_tensor("trib", [W, XP], bf16, kind="ExternalInput").ap()
    cidx = nc.dram_tensor("cidx", [4, 128, NB], i32, kind="ExternalInput").ap()
    iar = nc.dram_tensor("iar", [128, NB], f32, kind="ExternalInput").ap()
    mask = nc.dram_tensor("mask", [128, NCORES], f32, kind="ExternalInput").ap()
    pred = nc.dram_tensor("pred", [128 * NB, K], f32, kind="ExternalOutput").ap()

    RG = [list(range(NCORES))]
    NF = HSH * K

    with tile.TileContext(nc) as tc:
        with (
            tc.tile_pool(name="constp", bufs=1) as constp,
            tc.tile_pool(name="fmp", bufs=3) as fmp,
            tc.tile_pool(name="gp", bufs=3) as gp,
            tc.tile_pool(name="qp", bufs=1) as qp,
            tc.tile_pool(name="psp", bufs=2, space="PSUM") as psp,
            tc.tile_pool(name="gatp", bufs=12) as gatp,
            tc.tile_pool(name="xbp", bufs=1) as xbp,
            tc.tile_pool(name="dramp", bufs=1, space="DRAM") as dramp,
        ):
            # ---- constants -------------------------------------------------
            fcw_sb = constp.tile([128, CCH * K], bf16, tag="fcw", name="fcw_sb")
            nc.sync.dma_start(fcw_sb[:], fcw.rearrange("p cc k -> p (cc k)"))

            tri_w = []                       # [wsz, XP] per w-chunk
            for j, (off, sz) in enumerate(wch):
                t = constp.tile([sz, XP], bf16, tag=f"tri_w{j}", name=f"tri_w{j}")
                nc.sync.dma_start(t[:], trib[off:off + sz, :])
                tri_w.append(t)

            idx_sb = constp.tile([128, 4 * NB], i32, tag="idx", name="idx_sb")
            nc.sync.dma_start(idx_sb[:], cidx.rearrange("c p m -> p c m"))
            iar_sb = constp.tile([128, NB], f32, tag="iar", name="iar_sb")
            nc.sync.dma_start(iar_sb[:], iar[:, :])
            mask_sb = constp.tile([128, NCORES], f32, tag="mask", name="mask_sb")
            nc.sync.dma_start(mask_sb[:], mask[:, :])

            # persistent Q-cumsum buffers [x, (h k)]
            Qc = [qp.tile([sz, NF], f32, tag=f"Qc{j}", name=f"Qc{j}")
                  for j, (off, sz) in enumerate(xch)]

            # warm up the collective path early so the first real collective
            # doesn't pay the ncfw entry latency
            warm_in = dramp.tile([128, 4], f32, tag="warm_in", name="warm_in")
            warm_out = dramp.tile([128, 4], f32, tag="warm_out",
                                  name="warm_out", addr_space="Shared")
            wsb = constp.tile([128, 4], f32, tag="warm_sb", name="warm_sb")
            nc.gpsimd.memset(wsb[:], 0.0)
            nc.sync.dma_start(warm_in[:, :], wsb[:])
            nc.gpsimd.collective_compute(
                "AllReduce", mybir.AluOpType.add, replica_groups=RG,
                ins=[warm_in[:].opt()], outs=[warm_out[:].opt()],
            )

            # ---- phase 1: projection + W-cumsum + fused H-cumsum -----------
            for hq in range(HQ):
                fmh = fmp.tile([128, CCH * HR * W], bf16, tag="fmh", name="fmh")
                nc.sync.dma_start(fmh[:], fm.rearrange("p hq cc hr w -> p hq (cc hr w)")[:, hq])
                for hr in range(HR):
                    h = hq * HR + hr
                    gts = []
                    for wj, (woff, wsz) in enumerate(wch):
                        ps = psp.tile([wsz, K], f32, tag=f"pp{wj}", name="ps1")
                        for cc in range(CCH):
                            o = cc * (HR * W) + hr * W + woff
                            nc.tensor.matmul(
                                ps[:],
                                lhsT=fmh[:, o: o + wsz],
                                rhs=fcw_sb[:, cc * K:(cc + 1) * K],
                                start=(cc == 0), stop=(cc == CCH - 1),
                            )
                        gt = gp.tile([wsz, K], bf16, tag=f"g{wj}", name=f"g{wj}")
                        nc.vector.tensor_copy(gt[:], ps[:])
                        gts.append(gt)
                    for xj, (xoff, xsz) in enumerate(xch):
                        qs = psp.tile([xsz, K], f32, tag=f"wp{xj}", name="ps2")
                        for wj in range(len(wch)):
                            nc.tensor.matmul(
                                qs[:],
                                lhsT=tri_w[wj][:, xoff:xoff + xsz],
                                rhs=gts[wj][:],
                                start=(wj == 0), stop=(wj == len(wch) - 1),
                            )
                        if h == 0:
                            nc.vector.tensor_copy(Qc[xj][:, 0:K], qs[:])
                        else:
                            nc.vector.tensor_add(
                                Qc[xj][:, h * K:(h + 1) * K], qs[:],
                                Qc[xj][:, (h - 1) * K:h * K])

            # ---- block totals exchange (replicated AllToAll = AllGather) ---
            ag2_in = dramp.tile([NCORES * XP, K], f32, tag="ag2_in",
                                name="ag2_in")
            ag2_out = dramp.tile([NCORES * XP, K], f32, tag="ag2_out",
                                 name="ag2_out")
            a2iv = ag2_in.rearrange("(b x) k -> x b k", b=NCORES)
            for xj, (xoff, xsz) in enumerate(xch):
                src = (Qc[xj][:, (HSH - 1) * K:HSH * K]
                       .unsqueeze(1).broadcast_to([xsz, NCORES, K]))
                nc.sync.dma_start(a2iv[xoff:xoff + xsz], src)
            nc.gpsimd.collective_compute(
                "AllToAll", mybir.AluOpType.bypass, replica_groups=RG,
                ins=[ag2_in[:].opt()], outs=[ag2_out[:].opt()],
            )
            a2v = ag2_out.rearrange("(b x) k -> x b k", b=NCORES)
            P = []
            for xj, (xoff, xsz) in enumerate(xch):
                tall = qp.tile([xsz, NCORES * K], f32, tag=f"tall{xj}",
                               name=f"tall{xj}")
                nc.sync.dma_start(
                    tall[:].rearrange("x (b k) -> x b k", b=NCORES),
                    a2v[xoff:xoff + xsz])
                # tree-structured masked sum: 4 fused ops + 3 adds
                pairs = []
                for b in range(0, NCORES, 2):
                    t = qp.tile([xsz, K], f32, tag=f"Pt{xj}_{b}",
                                name=f"Pt{xj}_{b}")
                    nc.vector.tensor_scalar_mul(t[:], tall[:, b * K:(b + 1) * K],
                                                mask_sb[0:xsz, b:b + 1])
                    nc.vector.scalar_tensor_tensor(
                        out=t[:], in0=tall[:, (b + 1) * K:(b + 2) * K],
                        scalar=mask_sb[0:xsz, b + 1:b + 2], in1=t[:],
                        op0=mybir.AluOpType.mult, op1=mybir.AluOpType.add,
                    )
                    pairs.append(t)
                nc.vector.tensor_add(pairs[0][:], pairs[0][:], pairs[1][:])
                nc.vector.tensor_add(pairs[2][:], pairs[2][:], pairs[3][:])
                pfx = qp.tile([xsz, K], f32, tag=f"P{xj}", name=f"P{xj}")
                nc.vector.tensor_add(pfx[:], pairs[0][:], pairs[2][:])
                P.append(pfx)

            # ---- prefix add + store S chunks + chunked AllGather -----------
            XA = W + 1                       # 193 shipped x rows (x<=192)
            ag_ins = [dramp.tile([XA, CH[c] * K], f32, tag=f"ag_in{c}",
                                 name=f"ag_in{c}") for c in range(nch)]
            ag_outs = [dramp.tile([NCORES * XA * CH[c], K], f32,
                                  tag=f"ag_out{c}", name=f"ag_out{c}",
                                  addr_space="Shared") for c in range(nch)]
            for c in range(nch):
                for xj, (xoff, xsz) in enumerate(xch):
                    ssz = min(xsz, XA - xoff)
                    qv = Qc[xj][:, HO[c] * K:(HO[c] + CH[c]) * K]
                    pb = P[xj][:].unsqueeze(1).broadcast_to([xsz, CH[c], K])
                    nc.vector.tensor_add(
                        qv.rearrange("x (h k) -> x h k", k=K),
                        qv.rearrange("x (h k) -> x h k", k=K), pb)
                    nc.sync.dma_start(ag_ins[c][xoff:xoff + ssz, :],
                                      Qc[xj][0:ssz, HO[c] * K:(HO[c] + CH[c]) * K])
                nc.gpsimd.collective_compute(
                    "AllGather", mybir.AluOpType.bypass, replica_groups=RG,
                    ins=[ag_ins[c][:].opt()], outs=[ag_outs[c][:].opt()],
                )

            # ---- corner gathers + combine ----------------------------------
            # plan = (NB_SB, NB_XB): SB batches gather all 4 corners from the
            # LOCAL prefixed block (ag_in) — they run during the AllGather.
            # XB batches gather y1-corners (j0/j2) locally (early) and
            # y0-corners (j1/j3) from ag_out (after the AllGather).
            NB_SB, NB_L1, NB_L0, NB_G = plan
            loc_rows = ag_ins[0].rearrange("x (h k) -> (x h) k", k=K)
            pv = pred.rearrange("(m p) k -> p m k", p=128)

            def gather(tile_ap, src, col):
                nc.gpsimd.indirect_dma_start(
                    out=tile_ap, out_offset=None, in_=src,
                    in_offset=bass.IndirectOffsetOnAxis(
                        ap=idx_sb[:, col:col + 1], axis=0))

            def combine(g, m):
                nc.vector.tensor_sub(g[0][:], g[0][:], g[1][:])
                nc.vector.tensor_sub(g[2][:], g[2][:], g[3][:])
                nc.vector.tensor_sub(g[0][:], g[0][:], g[2][:])
                nc.vector.tensor_scalar_mul(g[1][:], g[0][:],
                                            iar_sb[:, m:m + 1])
                nc.sync.dma_start(pv[:, m, :], g[1][:])

            for m in range(NB_SB):              # all-local batches, early
                g = []
                for ci in range(4):
                    gt = gatp.tile([128, K], f32, tag=f"gt{ci}", name=f"gt{ci}")
                    gather(gt[:], loc_rows, ci * NB + m)
                    g.append(gt)
                combine(g, m)
            # L1: y1-corners (j0/j2) local early; L0: y0-corners (j1/j3)
            early = {}
            for m in range(NB_SB, NB_SB + NB_L1):
                for ci in (0, 2):
                    gt = xbp.tile([128, K], f32, tag=f"xb{ci}_{m}",
                                  name=f"xb{ci}_{m}")
                    gather(gt[:], loc_rows, ci * NB + m)
                    early[(m, ci)] = gt
            for m in range(NB_SB + NB_L1, NB_SB + NB_L1 + NB_L0):
                for ci in (1, 3):
                    gt = xbp.tile([128, K], f32, tag=f"xb{ci}_{m}",
                                  name=f"xb{ci}_{m}")
                    gather(gt[:], loc_rows, ci * NB + m)
                    early[(m, ci)] = gt
            # late gathers from the allgathered image + combines
            for m in range(NB_SB, NB):
                if m < NB_SB + NB_L1:
                    late_ci = (1, 3)
                elif m < NB_SB + NB_L1 + NB_L0:
                    late_ci = (0, 2)
                else:
                    late_ci = (0, 1, 2, 3)
                g = [None] * 4
                for ci in late_ci:
                    gt = gatp.tile([128, K], f32, tag=f"gl{ci}", name=f"gl{ci}")
                    gather(gt[:], ag_outs[0][:], ci * NB + m)
                    g[ci] = gt
                for ci in (0, 1, 2, 3):
                    if g[ci] is None:
                        g[ci] = early[(m, ci)]
                combine(g, m)

    nc.compile()
    return nc


def _get_nc(CH, plan):
    key = (tuple(CH), tuple(plan))
    if key not in _NC_CACHE:
        _NC_CACHE[key] = _build_nc(list(CH), list(plan))
    return _NC_CACHE[key]


def _prepare(feature_map, scale, anchors, fc_w, anchor_num, CH):
    """Host-side prep: swizzle fm, tri, per-core grouped corner plan."""
    import ml_dtypes
    bf = ml_dtypes.bfloat16

    N = int(anchor_num)
    assert N == N_ANCH, N
    anchors = np.asarray(anchors, dtype=np.float32)[:N]
    x0, x1, y0, y1 = _box_indices_np(anchors, np.asarray(scale, np.float32), H, W)
    area = np.maximum((y1 - y0) * (x1 - x0), 1).astype(np.float32)
    inv_area = (np.float32(1.0) / area).astype(np.float32)

    assert len(CH) == 1 and CH[0] == HSH
    XA = W + 1
    blk1, hh1 = (y1 - 1) // HSH, (y1 - 1) % HSH
    y0c = np.maximum(y0, 1)
    blk0, hh0 = (y0c - 1) // HSH, (y0c - 1) % HSH

    def lrid(x, hh, zero):
        # row id within the local prefixed block ag_in, viewed [XA*HSH, K]
        return np.where(zero, 0, x * HSH + hh).astype(np.int32)

    def grid(blk, x, hh, zero):
        # row id within the allgathered integral image
        return np.where(zero, 0,
                        blk * (XA * HSH) + x * HSH + hh).astype(np.int32)

    f_ = np.zeros_like(x1, dtype=bool)
    # SB anchors: y0 in same block (or y0==0 -> zero row): all 4 local
    sb = (y0 == 0) | (blk0 == blk1)
    # capacity-capped greedy assignment; spill goes to an all-global section
    N_ = len(x1)
    cap = N_ // NCORES
    owner = np.full(N_, -1, dtype=np.int64)
    load = np.zeros(NCORES, dtype=np.int64)
    pool = []
    for a in np.nonzero(sb)[0]:
        b = blk1[a]
        if load[b] < cap:
            owner[a] = b
            load[b] += 1
        else:
            pool.append(a)
    for a in np.nonzero(~sb)[0]:
        b0_, b1_ = blk0[a], blk1[a]
        cands = [b for b in (b0_, b1_) if load[b] < cap]
        if cands:
            b = min(cands, key=lambda c: load[c])
            owner[a] = b
            load[b] += 1
        else:
            pool.append(a)
    for a in pool:
        b = int(np.argmin(load))
        owner[a] = b
        load[b] += 1
    is_sb = sb & (owner == blk1)
    is_l1 = (~sb) & (owner == blk1)
    is_l0 = (~sb) & (owner == blk0)
    is_g = ~(is_sb | is_l1 | is_l0)
    if int(os.environ.get("NMS_ALL_GLOBAL", "1")):
        # proven-fast uniform plan: contiguous anchor slices, all corners
        # gathered from the allgathered image after the collective
        owner = np.arange(N_) // (N_ // NCORES)
        is_sb = np.zeros(N_, dtype=bool)
        is_l1 = is_sb.copy()
        is_l0 = is_sb.copy()
        is_g = ~is_sb
    use_l1 = is_sb | is_l1             # y1-corners local on owner
    use_l0 = is_sb | is_l0             # y0-corners local on owner
    r0 = np.where(use_l1, lrid(x1, hh1, f_), grid(blk1, x1, hh1, f_))
    r2 = np.where(use_l1, lrid(x0, hh1, x0 == 0), grid(blk1, x0, hh1, x0 == 0))
    z1 = y0 == 0
    z3 = (x0 == 0) | (y0 == 0)
    r1 = np.where(use_l0, lrid(x1, hh0, z1), grid(blk0, x1, hh0, z1))
    r3 = np.where(use_l0, lrid(x0, hh0, z3), grid(blk0, x0, hh0, z3))
    corners = np.stack([r0, r1, r2, r3]).astype(np.int32)   # [4, N]

    fcwT = np.ascontiguousarray(fc_w.T.astype(bf))                 # [C, K]
    fcw_in = np.ascontiguousarray(
        fcwT.reshape(CCH, 128, K).transpose(1, 0, 2))
    tri = np.zeros((W, XP), dtype=np.float32)
    for x in range(1, W + 1):
        tri[0:x, x] = 1.0
    trib = tri.astype(bf)

    maskf = np.zeros((NCORES, 128, NCORES), dtype=np.float32)
    for i in range(NCORES):
        maskf[i, :, :i] = 1.0

    # per-core sections: SB / L1 / L0 / G, one SPMD plan padded to maxes
    per_core = []
    for i in range(NCORES):
        mine = owner == i
        per_core.append((np.nonzero(mine & is_sb)[0],
                         np.nonzero(mine & is_l1)[0],
                         np.nonzero(mine & is_l0)[0],
                         np.nonzero(mine & is_g)[0]))
    NB_SB = max((len(t[0]) + 127) // 128 for t in per_core)
    NB_L1 = max((len(t[1]) + 127) // 128 for t in per_core)
    NB_L0 = max((len(t[2]) + 127) // 128 for t in per_core)
    NB_G = max((len(t[3]) + 127) // 128 for t in per_core)
    plan = (NB_SB, NB_L1, NB_L0, NB_G)
    NB = sum(plan)

    in_maps = []
    slot_maps = []
    for i in range(NCORES):
        sec = per_core[i]
        cid = np.zeros((4, NB * 128), dtype=np.int32)
        ia = np.zeros(NB * 128, dtype=np.float32)
        slots = np.full(NB * 128, -1, dtype=np.int64)
        off = 0
        for si, idxs in enumerate(sec):
            gsl = slice(off, off + len(idxs))
            cid[:, gsl] = corners[:, idxs]
            ia[gsl] = inv_area[idxs]
            slots[gsl] = idxs
            off += plan[si] * 128
        # reshape to [4, NB, 128] -> [4, 128, NB] slot (m, p) = m*128+p
        cid = cid.reshape(4, NB, 128).transpose(0, 2, 1)
        ia2 = ia.reshape(NB, 128).T
        slot_maps.append(slots)

        fm_i = feature_map[:, i * HSH:(i + 1) * HSH, :].astype(bf)
        fm_i = fm_i.reshape(CCH, 128, HQ, HR, W).transpose(1, 2, 0, 3, 4)
        in_maps.append({
            "fm": np.ascontiguousarray(fm_i),
            "fcw": fcw_in,
            "trib": trib,
            "cidx": np.ascontiguousarray(cid),
            "iar": np.ascontiguousarray(ia2),
            "mask": np.ascontiguousarray(maskf[i]),
        })
    return in_maps, slot_maps, plan


def kernel(**inputs):
    global LAST_RESULTS
    feature_map = np.asarray(inputs["feature_map"], dtype=np.float32)
    scale = np.asarray(inputs["scale"], dtype=np.float32)
    anchors = np.asarray(inputs["anchors"], dtype=np.float32)
    fc_w = np.asarray(inputs["fc_w"], dtype=np.float32)
    fc_b = np.asarray(inputs["fc_b"], dtype=np.float32)
    anchor_num = int(np.asarray(inputs["anchor_num"]))

    import time
    CH = _chunk_list()
    t0 = time.time()
    in_maps, slot_maps, plan = _prepare(feature_map, scale, anchors, fc_w,
                                        anchor_num, CH)
    print(f"[kernel] host prep {time.time() - t0:.1f}s NB={len(plan)}", flush=True)
    t0 = time.time()
    nc = _get_nc(CH, plan)
    print(f"[kernel] bass build+schedule {time.time() - t0:.1f}s", flush=True)

    from concourse.bass_utils import run_bass_kernel_spmd
    trace = bool(int(os.environ.get("NMS_TRACE", "0")))
    t0 = time.time()
    res = run_bass_kernel_spmd(nc, in_maps, core_ids=list(range(NCORES)),
                               trace=trace)
    print(f"[kernel] compile+run {time.time() - t0:.1f}s", flush=True)
    LAST_RESULTS = res
    pred = np.empty((N_ANCH, K), dtype=np.float32)
    for i in range(NCORES):
        block = res.results[i]["pred"]          # [NB*128, K] grouped order
        slots = slot_maps[i]                    # global anchor ids
        valid = slots >= 0
        pred[slots[valid]] = block[valid]
    return (pred + fc_b[None, :].astype(np.float32)).astype(np.float32)


# revision 30
# speedup vs baseline: 1.6432x; 1.3023x over previous
"""Bass/Trainium2 kernel for nn_FC_Classifier (box-pooled FC classifier).

Math: pred[n,k] = (1/area_n) * sum_{(h,w) in box_n} (fc_w @ feature_map)[k,h,w] + fc_b[k]

Strategy (8 cores, one chip), v3:
  * Shard image rows h across cores (24 rows/core).  Phase 1 contracts
    channels (2048 -> 150) with matmuls (host-swizzled fm, contiguous DMA).
  * W-cumsum fused per image row via triangular matmul (bf16 tri, f32 PSUM).
  * H-cumsum fused into the PSUM->SBUF copies as a running add chain.
  * Tiny AllGather of per-block column totals + mask-weighted prefix sum.
  * Integral image AllGather split into h-chunks (separate Shared tensors);
    anchors are grouped so every corner-gather instruction reads exactly one
    chunk tensor, so gathers overlap the later AllGather chunks' wire time.
  * 4-corner indirect-DMA gathers, combine on DVE, scale by 1/area; bias on
    host.

Self-contained: only numpy + the concourse (Bass) runtime are imported.
"""

import os
import numpy as np

DS = 8.0
NCORES = 8
C, H, W, K, N_ANCH = 2048, 192, 192, 150, 16384
HSH = H // NCORES              # 24 image rows per core
XP = 200                       # x range of S (0..192 used), padded to 8*25
CCH = C // 128                 # 16 channel chunks
HQ = 12                        # fm DMA chunks (2 rows each)
HR = HSH // HQ

LAST_RESULTS = None  # BassKernelResults of the most recent run (for test.py)

_NC_CACHE = {}


def _chunks(total, size):
    return [(o, min(size, total - o)) for o in range(0, total, size)]


def _chunk_list():
    s = os.environ.get("NMS_AG_CHUNKS", "24")
    ch = [int(x) for x in s.split(",") if x]
    assert sum(ch) == HSH, ch
    return ch


def _box_indices_np(anchors, scale, h, w):
    # exact replica of reference._box_indices in numpy f32
    a = anchors.astype(np.float32) / np.float32(DS)
    x0 = (a[:, 0] * scale[1]).astype(np.int32)
    x1 = (a[:, 1] * scale[1]).astype(np.int32)
    y0 = (a[:, 2] * scale[0]).astype(np.int32)
    y1 = (a[:, 3] * scale[0]).astype(np.int32)
    eqy = y0 == y1
    y0, y1 = (
        np.where(eqy & (y0 != 0), y0 - 1, y0),
        np.where(eqy & (y0 == 0), y1 + 1, y1),
    )
    eqx = x0 == x1
    x0, x1 = (
        np.where(eqx & (x0 != 0), x0 - 1, x0),
        np.where(eqx & (x0 == 0), x1 + 1, x1),
    )
    y0, y1 = np.clip(y0, 0, h), np.clip(y1, 0, h)
    x0, x1 = np.clip(x0, 0, w), np.clip(x1, 0, w)
    return x0, x1, y0, y1


def _build_nc(CH, plan):
    """Build the SPMD Bass program (identical on all 8 cores).

    CH: h-chunk sizes for the big AllGather (sum = 24).
    plan: per gather batch, (a, b) = chunk tensor ids for the y1-corners
          (j0/j2) and y0-corners (j1/j3).
    """
    from concourse import bacc, mybir, tile
    import concourse.bass as bass

    f32 = mybir.dt.float32
    bf16 = mybir.dt.bfloat16
    i32 = mybir.dt.int32

    wch = _chunks(W, 128)          # [(0,128),(128,64)]   w partition chunks
    xch = _chunks(XP, 128)         # [(0,128),(128,72)]   x partition chunks
    HO = np.concatenate([[0], np.cumsum(CH)]).astype(int)
    nch = len(CH)
    NB = sum(plan)

    nc = bacc.Bacc("TRN2", target_bir_lowering=False, debug=False,
                   num_devices=NCORES)
    # host-swizzled fm: [p, hq, cc, hr, w] so each h-chunk DMA is contiguous
    fm = nc.dram_tensor("fm", [128, HQ, CCH, HR, W], bf16, kind="ExternalInput").ap()
    fcw = nc.dram_tensor("fcw", [128, CCH, K], bf16, kind="ExternalInput").ap()
    trib = nc.dram_tensor("trib", [W, XP], bf16, kind="ExternalInput").ap()
    cidx = nc.dram_tensor("cidx", [4, 128, NB], i32, kind="ExternalInput").ap()
    iar = nc.dram_tensor("iar", [128, NB], f32, kind="ExternalInput").ap()
    mask = nc.dram_tensor("mask", [128, NCORES], f32, kind="ExternalInput").ap()
    pred = nc.dram_tensor("pred", [128 * NB, K], f32, kind="ExternalOutput").ap()

    RG = [list(range(NCORES))]
    NF = HSH * K

    with tile.TileContext(nc) as tc:
        with (
            tc.tile_pool(name="constp", bufs=1) as constp,
            tc.tile_pool(name="fmp", bufs=3) as fmp,
            tc.tile_pool(name="gp", bufs=3) as gp,
            tc.tile_pool(name="qp", bufs=1) as qp,
            tc.tile_pool(name="psp", bufs=2, space="PSUM") as psp,
            tc.tile_pool(name="gatp", bufs=12) as gatp,
            tc.tile_pool(name="xbp", bufs=1) as xbp,
            tc.tile_pool(name="dramp", bufs=1, space="DRAM") as dramp,
        ):
            # ---- constants -------------------------------------------------
            fcw_sb = constp.tile([128, CCH * K], bf16, tag="fcw", name="fcw_sb")
            nc.sync.dma_start(fcw_sb[:], fcw.rearrange("p cc k -> p (cc k)"))

            tri_w = []                       # [wsz, XP] per w-chunk
            for j, (off, sz) in enumerate(wch):
                t = constp.tile([sz, XP], bf16, tag=f"tri_w{j}", name=f"tri_w{j}")
                nc.sync.dma_start(t[:], trib[off:off + sz, :])
                tri_w.append(t)

            idx_sb = constp.tile([128, 4 * NB], i32, tag="idx", name="idx_sb")
            nc.sync.dma_start(idx_sb[:], cidx.rearrange("c p m -> p c m"))
            iar_sb = constp.tile([128, NB], f32, tag="iar", name="iar_sb")
            nc.sync.dma_start(iar_sb[:], iar[:, :])
            mask_sb = constp.tile([128, NCORES], f32, tag="mask", name="mask_sb")
            nc.sync.dma_start(mask_sb[:], mask[:, :])

            # persistent Q-cumsum buffers [x, (h k)]
            Qc = [qp.tile([sz, NF], f32, tag=f"Qc{j}", name=f"Qc{j}")
                  for j, (off, sz) in enumerate(xch)]

            # warm up the collective path early so the first real collective
            # doesn't pay the ncfw entry latency
            warm_in = dramp.tile([128, 4], f32, tag="warm_in", name="warm_in")
            warm_out = dramp.tile([128, 4], f32, tag="warm_out",
                                  name="warm_out", addr_space="Shared")
            wsb = constp.tile([128, 4], f32, tag="warm_sb", name="warm_sb")
            nc.gpsimd.memset(wsb[:], 0.0)
            nc.sync.dma_start(warm_in[:, :], wsb[:])
            nc.gpsimd.collective_compute(
                "AllReduce", mybir.AluOpType.add, replica_groups=RG,
                ins=[warm_in[:].opt()], outs=[warm_out[:].opt()],
            )

            # ---- phase 1: projection + W-cumsum + fused H-cumsum -----------
            for hq in range(HQ):
                fmh = fmp.tile([128, CCH * HR * W], bf16, tag="fmh", name="fmh")
                nc.sync.dma_start(fmh[:], fm.rearrange("p hq cc hr w -> p hq (cc hr w)")[:, hq])
                for hr in range(HR):
                    h = hq * HR + hr
                    gts = []
                    for wj, (woff, wsz) in enumerate(wch):
                        ps = psp.tile([wsz, K], f32, tag=f"pp{wj}", name="ps1")
                        for cc in range(CCH):
                            o = cc * (HR * W) + hr * W + woff
                            nc.tensor.matmul(
                                ps[:],
                                lhsT=fmh[:, o: o + wsz],
                                rhs=fcw_sb[:, cc * K:(cc + 1) * K],
                                start=(cc == 0), stop=(cc == CCH - 1),
                            )
                        gt = gp.tile([wsz, K], bf16, tag=f"g{wj}", name=f"g{wj}")
                        nc.vector.tensor_copy(gt[:], ps[:])
                        gts.append(gt)
                    for xj, (xoff, xsz) in enumerate(xch):
                        qs = psp.tile([xsz, K], f32, tag=f"wp{xj}", name="ps2")
                        for wj in range(len(wch)):
                            nc.tensor.matmul(
                                qs[:],
                                lhsT=tri_w[wj][:, xoff:xoff + xsz],
                                rhs=gts[wj][:],
                                start=(wj == 0), stop=(wj == len(wch) - 1),
                            )
                        if h == 0:
                            nc.vector.tensor_copy(Qc[xj][:, 0:K], qs[:])
                        else:
                            nc.vector.tensor_add(
                                Qc[xj][:, h * K:(h + 1) * K], qs[:],
                                Qc[xj][:, (h - 1) * K:h * K])

            # ---- block totals exchange (replicated AllToAll = AllGather) ---
            ag2_in = dramp.tile([NCORES * XP, K], f32, tag="ag2_in",
                                name="ag2_in")
            ag2_out = dramp.tile([NCORES * XP, K], f32, tag="ag2_out",
                                 name="ag2_out")
            a2iv = ag2_in.rearrange("(b x) k -> x b k", b=NCORES)
            for xj, (xoff, xsz) in enumerate(xch):
                src = (Qc[xj][:, (HSH - 1) * K:HSH * K]
                       .unsqueeze(1).broadcast_to([xsz, NCORES, K]))
                nc.sync.dma_start(a2iv[xoff:xoff + xsz], src)
            nc.gpsimd.collective_compute(
                "AllToAll", mybir.AluOpType.bypass, replica_groups=RG,
                ins=[ag2_in[:].opt()], outs=[ag2_out[:].opt()],
            )
            a2v = ag2_out.rearrange("(b x) k -> x b k", b=NCORES)
            P = []
            for xj, (xoff, xsz) in enumerate(xch):
                tall = qp.tile([xsz, NCORES * K], f32, tag=f"tall{xj}",
                               name=f"tall{xj}")
                nc.sync.dma_start(
                    tall[:].rearrange("x (b k) -> x b k", b=NCORES),
                    a2v[xoff:xoff + xsz])
                # tree-structured masked sum: 4 fused ops + 3 adds
                pairs = []
                for b in range(0, NCORES, 2):
                    t = qp.tile([xsz, K], f32, tag=f"Pt{xj}_{b}",
                                name=f"Pt{xj}_{b}")
                    nc.vector.tensor_scalar_mul(t[:], tall[:, b * K:(b + 1) * K],
                                                mask_sb[0:xsz, b:b + 1])
                    nc.vector.scalar_tensor_tensor(
                        out=t[:], in0=tall[:, (b + 1) * K:(b + 2) * K],
                        scalar=mask_sb[0:xsz, b + 1:b + 2], in1=t[:],
                        op0=mybir.AluOpType.mult, op1=mybir.AluOpType.add,
                    )
                    pairs.append(t)
                nc.vector.tensor_add(pairs[0][:], pairs[0][:], pairs[1][:])
                nc.vector.tensor_add(pairs[2][:], pairs[2][:], pairs[3][:])
                pfx = qp.tile([xsz, K], f32, tag=f"P{xj}", name=f"P{xj}")
                nc.vector.tensor_add(pfx[:], pairs[0][:], pairs[2][:])
                P.append(pfx)

            # ---- prefix add + store S chunks + chunked AllGather -----------
            XA = W + 1                       # 193 shipped x rows (x<=192)
            ag_ins = [dramp.tile([XA, CH[c] * K], f32, tag=f"ag_in{c}",
                                 name=f"ag_in{c}") for c in range(nch)]
            ag_outs = [dramp.tile([NCORES * XA * CH[c], K], f32,
                                  tag=f"ag_out{c}", name=f"ag_out{c}",
                                  addr_space="Shared") for c in range(nch)]
            for c in range(nch):
                for xj, (xoff, xsz) in enumerate(xch):
                    ssz = min(xsz, XA - xoff)
                    qv = Qc[xj][:, HO[c] * K:(HO[c] + CH[c]) * K]
                    pb = P[xj][:].unsqueeze(1).broadcast_to([xsz, CH[c], K])
                    nc.vector.tensor_add(
                        qv.rearrange("x (h k) -> x h k", k=K),
                        qv.rearrange("x (h k) -> x h k", k=K), pb)
                    nc.sync.dma_start(ag_ins[c][xoff:xoff + ssz, :],
                                      Qc[xj][0:ssz, HO[c] * K:(HO[c] + CH[c]) * K])
                nc.gpsimd.collective_compute(
                    "AllGather", mybir.AluOpType.bypass, replica_groups=RG,
                    ins=[ag_ins[c][:].opt()], outs=[ag_outs[c][:].opt()],
                )

            # ---- corner gathers + combine ----------------------------------
            # plan = (NB_SB, NB_XB): SB batches gather all 4 corners from the
            # LOCAL prefixed block (ag_in) — they run during the AllGather.
            # XB batches gather y1-corners (j0/j2) locally (early) and
            # y0-corners (j1/j3) from ag_out (after the AllGather).
            NB_SB, NB_L1, NB_L0, NB_G = plan
            loc_rows = ag_ins[0].rearrange("x (h k) -> (x h) k", k=K)
            pv = pred.rearrange("(m p) k -> p m k", p=128)

            def gather(tile_ap, src, col):
                nc.gpsimd.indirect_dma_start(
                    out=tile_ap, out_offset=None, in_=src,
                    in_offset=bass.IndirectOffsetOnAxis(
                        ap=idx_sb[:, col:col + 1], axis=0))

            def combine(g, m):
                nc.vector.tensor_sub(g[0][:], g[0][:], g[1][:])
                nc.vector.tensor_sub(g[2][:], g[2][:], g[3][:])
                nc.vector.tensor_sub(g[0][:], g[0][:], g[2][:])
                nc.vector.tensor_scalar_mul(g[1][:], g[0][:],
                                            iar_sb[:, m:m + 1])
                nc.sync.dma_start(pv[:, m, :], g[1][:])

            for m in range(NB_SB):              # all-local batches, early
                g = []
                for ci in range(4):
                    gt = gatp.tile([128, K], f32, tag=f"gt{ci}", name=f"gt{ci}")
                    gather(gt[:], loc_rows, ci * NB + m)
                    g.append(gt)
                combine(g, m)
            # L1: y1-corners (j0/j2) local early; L0: y0-corners (j1/j3)
            early = {}
            for m in range(NB_SB, NB_SB + NB_L1):
                for ci in (0, 2):
                    gt = xbp.tile([128, K], f32, tag=f"xb{ci}_{m}",
                                  name=f"xb{ci}_{m}")
                    gather(gt[:], loc_rows, ci * NB + m)
                    early[(m, ci)] = gt
            for m in range(NB_SB + NB_L1, NB_SB + NB_L1 + NB_L0):
                for ci in (1, 3):
                    gt = xbp.tile([128, K], f32, tag=f"xb{ci}_{m}",
                                  name=f"xb{ci}_{m}")
                    gather(gt[:], loc_rows, ci * NB + m)
                    early[(m, ci)] = gt
            # late gathers from the allgathered image + combines
            for m in range(NB_SB, NB):
                if m < NB_SB + NB_L1:
                    late_ci = (1, 3)
                elif m < NB_SB + NB_L1 + NB_L0:
                    late_ci = (0, 2)
                else:
                    late_ci = (0, 1, 2, 3)
                g = [None] * 4
                for ci in late_ci:
                    gt = gatp.tile([128, K], f32, tag=f"gl{ci}", name=f"gl{ci}")
                    gather(gt[:], ag_outs[0][:], ci * NB + m)
                    g[ci] = gt
                for ci in (0, 1, 2, 3):
                    if g[ci] is None:
                        g[ci] = early[(m, ci)]
                combine(g, m)

    nc.compile()
    return nc


def _get_nc(CH, plan):
    key = (tuple(CH), tuple(plan))
    if key not in _NC_CACHE:
        _NC_CACHE[key] = _build_nc(list(CH), list(plan))
    return _NC_CACHE[key]


def _prepare(feature_map, scale, anchors, fc_w, anchor_num, CH):
    """Host-side prep: swizzle fm, tri, per-core grouped corner plan."""
    import ml_dtypes
    bf = ml_dtypes.bfloat16

    N = int(anchor_num)
    assert N == N_ANCH, N
    anchors = np.asarray(anchors, dtype=np.float32)[:N]
    x0, x1, y0, y1 = _box_indices_np(anchors, np.asarray(scale, np.float32), H, W)
    area = np.maximum((y1 - y0) * (x1 - x0), 1).astype(np.float32)
    inv_area = (np.float32(1.0) / area).astype(np.float32)

    assert len(CH) == 1 and CH[0] == HSH
    XA = W + 1
    blk1, hh1 = (y1 - 1) // HSH, (y1 - 1) % HSH
    y0c = np.maximum(y0, 1)
    blk0, hh0 = (y0c - 1) // HSH, (y0c - 1) % HSH

    def lrid(x, hh, zero):
        # row id within the local prefixed block ag_in, viewed [XA*HSH, K]
        return np.where(zero, 0, x * HSH + hh).astype(np.int32)

    def grid(blk, x, hh, zero):
        # row id within the allgathered integral image
        return np.where(zero, 0,
                        blk * (XA * HSH) + x * HSH + hh).astype(np.int32)

    f_ = np.zeros_like(x1, dtype=bool)
    # SB anchors: y0 in same block (or y0==0 -> zero row): all 4 local
    sb = (y0 == 0) | (blk0 == blk1)
    # balanced fixed-size sections: every core gets T_SB all-local batches,
    # T_L0 batches with y0-corners local (feasible: surplus same-block
    # anchors qualify as their own L0; y0==0 zero-corners are local on any
    # core), and the leftovers as all-global G batches.  Zero padding.
    N_ = len(x1)
    T_SB, T_L0 = 3, 10
    taken = np.zeros(N_, dtype=bool)
    per_sb, per_l0 = [], []
    for i in range(NCORES):
        c = np.nonzero(sb & (blk1 == i) & ~taken)[0][:T_SB * 128]
        taken[c] = True
        per_sb.append(c)
    for i in range(NCORES):
        c = np.nonzero(~taken & ((y0 == 0) | (blk0 == i)))[0][:T_L0 * 128]
        taken[c] = True
        per_l0.append(c)
    left = np.nonzero(~taken)[0]
    per_g = [left[j::NCORES] for j in range(NCORES)]
    NB_G = max((len(g) + 127) // 128 for g in per_g)
    plan = (T_SB, 0, T_L0, NB_G)
    NB = sum(plan)
    per_core = [(per_sb[i], np.array([], dtype=np.int64), per_l0[i],
                 per_g[i]) for i in range(NCORES)]

    use_l1 = np.zeros(N_, dtype=bool)
    use_l0 = np.zeros(N_, dtype=bool)
    for i in range(NCORES):
        use_l1[per_sb[i]] = True
        use_l0[per_sb[i]] = True
        use_l0[per_l0[i]] = True
    r0 = np.where(use_l1, lrid(x1, hh1, f_), grid(blk1, x1, hh1, f_))
    r2 = np.where(use_l1, lrid(x0, hh1, x0 == 0), grid(blk1, x0, hh1, x0 == 0))
    z1 = y0 == 0
    z3 = (x0 == 0) | (y0 == 0)
    r1 = np.where(use_l0, lrid(x1, hh0, z1), grid(blk0, x1, hh0, z1))
    r3 = np.where(use_l0, lrid(x0, hh0, z3), grid(blk0, x0, hh0, z3))
    corners = np.stack([r0, r1, r2, r3]).astype(np.int32)   # [4, N]

    fcwT = np.ascontiguousarray(fc_w.T.astype(bf))                 # [C, K]
    fcw_in = np.ascontiguousarray(
        fcwT.reshape(CCH, 128, K).transpose(1, 0, 2))
    tri = np.zeros((W, XP), dtype=np.float32)
    for x in range(1, W + 1):
        tri[0:x, x] = 1.0
    trib = tri.astype(bf)

    maskf = np.zeros((NCORES, 128, NCORES), dtype=np.float32)
    for i in range(NCORES):
        maskf[i, :, :i] = 1.0

    in_maps = []
    slot_maps = []
    for i in range(NCORES):
        sec = per_core[i]
        cid = np.zeros((4, NB * 128), dtype=np.int32)
        ia = np.zeros(NB * 128, dtype=np.float32)
        slots = np.full(NB * 128, -1, dtype=np.int64)
        off = 0
        for si, idxs in enumerate(sec):
            gsl = slice(off, off + len(idxs))
            cid[:, gsl] = corners[:, idxs]
            ia[gsl] = inv_area[idxs]
            slots[gsl] = idxs
            off += plan[si] * 128
        # reshape to [4, NB, 128] -> [4, 128, NB] slot (m, p) = m*128+p
        cid = cid.reshape(4, NB, 128).transpose(0, 2, 1)
        ia2 = ia.reshape(NB, 128).T
        slot_maps.append(slots)

        fm_i = feature_map[:, i * HSH:(i + 1) * HSH, :].astype(bf)
        fm_i = fm_i.reshape(CCH, 128, HQ, HR, W).transpose(1, 2, 0, 3, 4)
        in_maps.append({
            "fm": np.ascontiguousarray(fm_i),
            "fcw": fcw_in,
            "trib": trib,
            "cidx": np.ascontiguousarray(cid),
            "iar": np.ascontiguousarray(ia2),
            "mask": np.ascontiguousarray(maskf[i]),
        })
    return in_maps, slot_maps, plan


def kernel(**inputs):
    global LAST_RESULTS
    feature_map = np.asarray(inputs["feature_map"], dtype=np.float32)
    scale = np.asarray(inputs["scale"], dtype=np.float32)
    anchors = np.asarray(inputs["anchors"], dtype=np.float32)
    fc_w = np.asarray(inputs["fc_w"], dtype=np.float32)
    fc_b = np.asarray(inputs["fc_b"], dtype=np.float32)
    anchor_num = int(np.asarray(inputs["anchor_num"]))

    import time
    CH = _chunk_list()
    t0 = time.time()
    in_maps, slot_maps, plan = _prepare(feature_map, scale, anchors, fc_w,
                                        anchor_num, CH)
    print(f"[kernel] host prep {time.time() - t0:.1f}s NB={len(plan)}", flush=True)
    t0 = time.time()
    nc = _get_nc(CH, plan)
    print(f"[kernel] bass build+schedule {time.time() - t0:.1f}s", flush=True)

    from concourse.bass_utils import run_bass_kernel_spmd
    trace = bool(int(os.environ.get("NMS_TRACE", "0")))
    t0 = time.time()
    res = run_bass_kernel_spmd(nc, in_maps, core_ids=list(range(NCORES)),
                               trace=trace)
    print(f"[kernel] compile+run {time.time() - t0:.1f}s", flush=True)
    LAST_RESULTS = res
    pred = np.empty((N_ANCH, K), dtype=np.float32)
    for i in range(NCORES):
        block = res.results[i]["pred"]          # [NB*128, K] grouped order
        slots = slot_maps[i]                    # global anchor ids
        valid = slots >= 0
        pred[slots[valid]] = block[valid]
    return (pred + fc_b[None, :].astype(np.float32)).astype(np.float32)
